# revision 1
# baseline (speedup 1.0000x reference)
"""Trainium2 Bass kernel for nn_CP2_17669495456475 (dynamic-kernel deconv).

Math: out[b,c,y,x] = sum_l cos[b,l,i,j] * W[b,l,c,ky,kx],  y=8i+ky, x=8j+kx,
with W = unfold(pad(b)) * (1 - unfold(pad(mask))), K=16, S=8, crop 4.

Decomposition (per core): since K = 2*S, split ky = ry + 8*sy, kx = rx + 8*sx.
With u = i+sy, v = j+sx the whole op is ONE matmul with contraction over
(a,sy,sx,p) -> (l,sy,sx) of size 4096:

  outT[(c,ry,rx), (u,v)] = sum_{l,sy,sx} bm_block[(li+sy, lj+sx), (c,ry,rx)]
                                          * Xp[l, 1+u-sy, 1+v-sx]

where bm = pad(b)*(1-pad(mask)) laid out in 8x8 blocks (the unfold becomes
duplication-free shifted block views) and the deconv overlap-add is absorbed
into PSUM accumulation.  The mask multiply is fused on-device (DVE) on the
gathered W chunk tiles.

Sharding: 8 cores = 4 batches x 2 channel-halves (16 ch each). Full inputs in,
full output out; host does layout glue (replicate pad, block reshape, zero pad,
final crop/assembly) only.
"""

import os
import numpy as np

import concourse.bass as bass
import concourse.mybir as mybir
import concourse.tile as tile
from concourse.bass_utils import run_bass_kernel_spmd

PD = 4
C = 16              # channels per core
N_CORES = 8
CHUNKS = [(a, sy, sx) for a in range(8) for sy in (0, 1) for sx in (0, 1)]

# matmul input dtype: "f32" (exact, 4 cyc/row), "f32r" (relaxed, 1 cyc/row,
# slow 4-byte weight loads), or "bf16" (1 cyc/row + fast weight load)
MM_DTYPE = os.environ.get("BASSK_MM_DTYPE", "bf16")
# fp32r requires an even innermost moving count -> keep the dead v=33 col;
# bf16/f32 have no such restriction.
NV = 34 if MM_DTYPE == "f32r" else 33
NT = 11 * NV          # N per matmul: 11 u-rows x NV v-cols


def _split_multi_sync(nc):
    """The walrus in this env allows only ONE sync-wait per instruction.
    Hoist extra waits onto same-engine InstNoOp carriers placed just before
    the owning instruction (sequential waits on one engine == AND)."""
    ctr = 0
    for f in nc.m.functions:
        for bb in f.blocks:
            insts = list(bb.instructions)
            out = []
            changed = False
            for inst in insts:
                si = inst.sync_info
                waits = list(si.on_wait) if si and si.on_wait else []
                if len(waits) > 1:
                    for w in waits[:-1]:
                        nop = mybir.InstNoOp(name=f"waitnop-{ctr}", ins=[], outs=[])
                        ctr += 1
                        nop.engine = inst.engine
                        nop.sync_info = mybir.SyncInfo(on_wait=[w], on_update=[])
                        out.append(nop)
                    si.on_wait = [waits[-1]]
                    changed = True
                out.append(inst)
            if changed:
                bb.instructions = out
    return ctr


def _build_nc():
    f32 = mybir.dt.float32
    # float32r has identical bits/np-dtype to float32; the PE runs its
    # matmuls at 1 cyc/row (vs 4 for exact f32). The BIR verifier requires
    # every producer of an f32r matmul operand to *output* f32r, so the
    # W/X dram params and sbuf tiles are typed f32r end-to-end.
    mmdt = {"f32r": mybir.dt.float32r, "bf16": mybir.dt.bfloat16}.get(MM_DTYPE, f32)
    nc = bass.Bass(enable_partition_id=False)
    # W chunks pre-gathered host-side, partition-major: [p, ci, (c,ry,rx)].
    # 16KB contiguous per partition per 4-chunk quad -> large DMA packets
    # (4KB runs cost ~400ns/packet overhead and cap DMA at ~160 GB/s).
    w4 = nc.declare_dram_parameter("w4", [128, 32, C * 64], mmdt, isOutput=False)
    mT = nc.declare_dram_parameter("mT", [128, 32, 64], f32, isOutput=False)
    # X is y-major [p, yy, a, xx] and loads in 3 phase-aligned y-slabs:
    # phase n only reads rows [11n, 11n+13), so the first matmuls need just
    # slab 0 (rows 0..13) instead of the whole 5MB tensor.
    xp = nc.declare_dram_parameter("xp", [128, 34, 8, 36], mmdt, isOutput=False)
    # a-major copy of slab-0 rows so the startup slivers are contiguous
    xp0 = nc.declare_dram_parameter("xp0", [128, 8, 13, 36], mmdt, isOutput=False)
    # out: phase-major [n, p, m, NT] so each phase writes one 12KB/partition DMA
    outT = nc.declare_dram_parameter("outT", [3, 128, 8, NT], f32, isOutput=True)

    with tile.TileContext(nc) as tc:
        with (
            tc.tile_pool(name="xpp", bufs=1) as xpp,
            tc.tile_pool(name="wp", bufs=8) as wp,
            tc.tile_pool(name="mp", bufs=1) as mp,
            tc.tile_pool(name="onp", bufs=4) as onp,
            tc.tile_pool(name="op", bufs=5) as op,
            tc.tile_pool(name="pp", bufs=8, space="PSUM") as pp,
        ):
            # Startup-critical DMAs lead each queue's FIFO: the first matmul
            # needs only s0a (a=0; chunks 0-3 all read a=0) + mta0 + chunk 0
            # of W on the sync queue.
            # one tile per y-slab (separate tiles: dependency tracking is
            # whole-tile, and deferred slabs must not deadlock phase-0 reads).
            # Slabs overlap by 2 rows so each phase reads within one slab.
            SLABS = ((0, 13), (11, 24), (22, 34))
            s0a = xpp.tile([128, 1, 13, 36], mmdt)
            nc.scalar.dma_start(s0a[:], xp0[:, 0:1])
            mta0 = mp.tile([128, 4, 64], f32)
            nc.scalar.dma_start(mta0[:], mT[:, 0:4])
            # mta0 := 1 - m in place, one op for all 4 chunks
            nc.vector.tensor_scalar(
                out=mta0[:], in0=mta0[:], scalar1=-1.0, scalar2=1.0,
                op0=mybir.AluOpType.mult, op1=mybir.AluOpType.add,
            )
            s0b = xpp.tile([128, 7, 13, 36], mmdt)
            nc.scalar.dma_start(s0b[:], xp0[:, 1:8])
            mta1 = mp.tile([128, 28, 64], f32)
            nc.scalar.dma_start(mta1[:], mT[:, 4:32])
            # mta1 := 1 - m in place, one op for the other 28 chunks
            nc.vector.tensor_scalar(
                out=mta1[:], in0=mta1[:], scalar1=-1.0, scalar2=1.0,
                op0=mybir.AluOpType.mult, op1=mybir.AluOpType.add,
            )

            def mask_ap(ci):
                return mta0[:, ci, :] if ci < 4 else mta1[:, ci - 4, :]
            slab_dmas = [None]
            slabs = [None]
            for si, (y0, y1) in list(enumerate(SLABS))[1:]:
                st = xpp.tile([128, y1 - y0, 8, 36], mmdt, name=f"slab_{si}")
                slab_dmas.append(nc.scalar.dma_start(st[:], xp[:, y0:y1]))
                slabs.append(st)

            def rhs_ap(n, a, sy, sx):
                y0 = 11 * n + 1 - sy - SLABS[n][0]
                x0 = 1 - sx
                if n == 0:
                    t, aa = (s0a, a) if a < 1 else (s0b, a - 1)
                    return t[:, aa, y0:y0 + 11, x0:x0 + NV]
                return slabs[n][:, y0:y0 + 11, a, x0:x0 + NV]

            # W streams in 8 quads of 4 chunks; mask-mul fused per chunk.
            # Chunk 0 gets its OWN tile: dependency tracking is whole-tile,
            # so as a quad slice its first matmul would wait on all 4 muls.
            wc0 = wp.tile([128, 1, C * 64], mmdt, name="wc0")
            nc.sync.dma_start(wc0[:], w4[:, 0:1, :])
            quads = []
            quad_dmas = []
            for g in range(8):
                wq = wp.tile([128, 4, C * 64], mmdt, tag="w", name=f"wq_{g}")
                if g == 0:
                    dma = nc.sync.dma_start(wq[:, 1:4, :], w4[:, 1:4, :])
                else:
                    dma = nc.sync.dma_start(wq[:], w4[:, 4 * g:4 * g + 4, :])
                quad_dmas.append(dma)
                for j in range(4):
                    ci = 4 * g + j
                    wt_dst = wc0 if ci == 0 else wq
                    jj = 0 if ci == 0 else j
                    wv = wt_dst[:, jj, :].rearrange("p (c f) -> p c f", c=C)
                    in0 = wv.bitcast(f32) if MM_DTYPE == "f32r" else wv
                    nc.vector.tensor_tensor(
                        out=wv, in0=in0,
                        in1=mask_ap(ci)[:, None, :].broadcast_to([128, C, 64]),
                        op=mybir.AluOpType.mult,
                    )
                quads.append(wq)

            def lhsT(ci, m):
                if ci == 0:
                    return wc0[:, 0, 128 * m:128 * (m + 1)]
                return quads[ci // 4][:, ci % 4, 128 * m:128 * (m + 1)]

            # Phase 0 (n=0) is chunk-outer with 8 live psum groups so the PE
            # consumes W quads as they stream (no all-32-chunks stall).
            # Phases 1-2 run m-outer (all data resident) so each group's
            # psum copy + output DMA overlaps the next group's matmuls.
            for n in range(3):
                # per-2m writeback tiles: dependency tracking is whole-tile,
                # so one big osb would hold every outT DMA until the LAST
                # psum copy; pair tiles let earlier halves fly mid-phase and
                # shrink the post-last-matmul tail to one 0.37MB DMA.
                def osb_pair(k):
                    return op.tile([128, 2, NT], f32, tag="o", name=f"osb_{n}_{k}")
                if n == 0:
                    pss = [pp.tile([128, NT], f32, tag="ps", name=f"ps_{n}_{i}")
                           for i in range(8)]
                    mm0 = {}
                    for ci, (a, sy, sx) in enumerate(CHUNKS):
                        rhs = rhs_ap(n, a, sy, sx)
                        for m in range(8):
                            mm0[ci, m] = nc.tensor.matmul(
                                pss[m][:], lhsT(ci, m), rhs,
                                start=(ci == 0), stop=(ci == 31),
                            )
                    # Defer the late W quads / X slabs behind PE progress so
                    # the startup-critical DMAs (mta, quad0, slab0) get the
                    # full DMA bandwidth during the ramp.
                    from concourse.tile_rust import add_dep_helper
                    add_dep_helper(quad_dmas[2].ins, mm0[0, 0].ins,
                                   sync=True, reason="stream quads behind PE")
                    for g in range(3, 8):
                        add_dep_helper(quad_dmas[g].ins, mm0[4 * (g - 3) + 3, 7].ins,
                                       sync=True, reason="stream quads behind PE")
                    add_dep_helper(slab_dmas[1].ins, mm0[3, 7].ins,
                                   sync=True, reason="slab1 after early phase0")
                    add_dep_helper(slab_dmas[2].ins, mm0[11, 7].ins,
                                   sync=True, reason="slab2 after mid phase0")
                    for k in range(4):
                        ot = osb_pair(k)
                        nc.vector.tensor_copy(ot[:, 0, :], pss[2 * k][:])
                        nc.vector.tensor_copy(ot[:, 1, :], pss[2 * k + 1][:])
                        nc.scalar.dma_start(outT[n, :, 2 * k:2 * k + 2], ot[:])
                else:
                    ot = None
                    for m in range(8):
                        ps = pp.tile([128, NT], f32, tag="ps", name=f"ps_{n}_{m}")
                        for ci, (a, sy, sx) in enumerate(CHUNKS):
                            rhs = rhs_ap(n, a, sy, sx)
                            nc.tensor.matmul(
                                ps[:], lhsT(ci, m), rhs,
                                start=(ci == 0), stop=(ci == 31),
                            )
                        if m % 2 == 0:
                            ot = osb_pair(m // 2)
                        nc.vector.tensor_copy(ot[:, m % 2, :], ps[:])
                        if m % 2 == 1:
                            nc.scalar.dma_start(
                                outT[n, :, m - 1:m + 1], ot[:])

    _split_multi_sync(nc)
    return nc


def _host_prep(b_ch, mask_b, cos_b):
    """b_ch (16,256,256) f32, mask_b (256,256) f32, cos_b (1024,32,32) f32
    -> dict of device inputs (layout/gather glue only)."""
    bpad = np.pad(b_ch, ((0, 0), (PD, PD), (PD, PD)), mode="edge")
    mpad = np.pad(mask_b, ((PD, PD), (PD, PD)), mode="edge")
    # block layout [bi*33+bj, (c,ry,rx)]
    bT = bpad.reshape(C, 33, 8, 33, 8).transpose(1, 3, 0, 2, 4).reshape(33 * 33, C * 64)
    mTb = mpad.reshape(33, 8, 33, 8).transpose(0, 2, 1, 3).reshape(33 * 33, 64)
    # unfold-as-shifted-block-views: chunk (a,sy,sx), partition p=32*pi+pj
    # reads block row (4a+pi+sy)*33 + (pj+sx).  Pre-gather partition-major.
    pi, pj = np.arange(4)[:, None], np.arange(32)[None, :]
    rows = np.stack([((4 * a + pi + sy) * 33 + (pj + sx)).reshape(128)
                     for (a, sy, sx) in CHUNKS], axis=1)        # [128, 32]
    w4 = np.ascontiguousarray(bT[rows])                          # [128,32,1024]
    mT = np.ascontiguousarray(mTb[rows])                         # [128,32,64]
    xp = np.zeros((1024, 34, 36), np.float32)
    xp[:, 1:33, 1:33] = cos_b
    # [l=128a+p, yy, xx] -> [p, yy, a, xx]; plus an a-major slab-0 copy
    xpb = xp.reshape(8, 128, 34, 36)
    xp0 = np.ascontiguousarray(xpb[:, :, 0:13, :].transpose(1, 0, 2, 3))
    xp = np.ascontiguousarray(xpb.transpose(1, 2, 0, 3))
    if MM_DTYPE == "bf16":
        import ml_dtypes
        w4 = w4.astype(ml_dtypes.bfloat16)
        xp = xp.astype(ml_dtypes.bfloat16)
        xp0 = xp0.astype(ml_dtypes.bfloat16)
    return {"w4": w4, "mT": mT, "xp": xp, "xp0": xp0}


def _unshard(outT):
    # outT [3, 128, 8, 11*NV] -> [(c,ry,rx)=128m+p, u=11n+u', v] -> (16,256,256)
    outT = np.asarray(outT, dtype=np.float32)
    t = outT.reshape(3, 128, 8, 11, NV).transpose(2, 1, 0, 3, 4).reshape(1024, 33, NV)
    t = t[:, :, :33].reshape(C, 8, 8, 33, 33).transpose(0, 3, 1, 4, 2)
    return t.reshape(C, 264, 264)[:, 4:260, 4:260]


_RUN_KW = {}   # test harness may inject e.g. trace=True
_LAST_RESULTS = [None]
_NC_CACHE = {}


def _get_nc():
    nc = _NC_CACHE.get(MM_DTYPE)
    if nc is None:
        nc = _NC_CACHE[MM_DTYPE] = _build_nc()
    return nc


def kernel(cos_similar, b, mask):
    cos_similar = np.ascontiguousarray(np.asarray(cos_similar, dtype=np.float32))
    b = np.ascontiguousarray(np.asarray(b, dtype=np.float32))
    mask = np.ascontiguousarray(np.asarray(mask, dtype=np.float32))

    in_maps = []
    for core in range(N_CORES):
        batch, half = core // 2, core % 2
        ch0 = C * half
        in_maps.append(_host_prep(
            b[batch, ch0:ch0 + C], mask[batch, 0], cos_similar[batch]))

    nc = _get_nc()
    res = run_bass_kernel_spmd(nc, in_maps, list(range(N_CORES)), **_RUN_KW)
    _LAST_RESULTS[0] = res

    out = np.empty((4, 32, 256, 256), np.float32)
    for core in range(N_CORES):
        batch, half = core // 2, core % 2
        ch0 = C * half
        out[batch, ch0:ch0 + C] = _unshard(res.results[core]["outT"])
    return out



# revision 3
# speedup vs baseline: 1.0421x; 1.0421x over previous
"""Trainium2 Bass kernel for nn_CP2_17669495456475 (dynamic-kernel deconv).

Math: out[b,c,y,x] = sum_l cos[b,l,i,j] * W[b,l,c,ky,kx],  y=8i+ky, x=8j+kx,
with W = unfold(pad(b)) * (1 - unfold(pad(mask))), K=16, S=8, crop 4.

Decomposition (per core): since K = 2*S, split ky = ry + 8*sy, kx = rx + 8*sx.
With u = i+sy, v = j+sx the whole op is ONE matmul with contraction over
(a,sy,sx,p) -> (l,sy,sx) of size 4096:

  outT[(c,ry,rx), (u,v)] = sum_{l,sy,sx} bm_block[(li+sy, lj+sx), (c,ry,rx)]
                                          * Xp[l, 1+u-sy, 1+v-sx]

where bm = pad(b)*(1-pad(mask)) laid out in 8x8 blocks (the unfold becomes
duplication-free shifted block views) and the deconv overlap-add is absorbed
into PSUM accumulation.  The mask multiply is fused on-device (DVE) on the
streamed W chunk tiles.

Sharding: 8 cores = 4 batches x 2 channel-halves (16 ch each). Full inputs in,
full output out; host does layout glue (replicate pad, block reshape, zero pad,
dtype cast, final crop/assembly) only.

Perf notes (v2): the NTFF profile of v1 showed the warm MM stream already at
~159ns per 363-col MM with zero stalls; all remaining time was (a) an 18.25us
startup (critical chain: mask DMA -> 1-m -> mask-mul -> first MM, behind
bulk DMA), (b) a cold-clock (HAM K=4/8) window around the first MMs, (c) the
tail.  v2:
 - tiny critical-path slivers (chunk-0 mask col, chunk-0 per-m W slices) at
   the head of the sync HWDGE queue, X/mask stream on the scalar queue, W
   streamed per-chunk (256KB FIFO granularity) so the early chunks land just
   in time -> first MM ~9.5us and no early-chunk starvation;
 - ~14 dependency-free warm-up matmuls on memset data bridge the PE busy
   window so the HAM clock gate opens before/just as the real stream begins;
 - masks in bf16 (2x DVE rate, halves mask DMA bytes);
 - border chunks whose rhs row is the all-zero X pad (sy=1 in phase 0, sy=0
   in phase 2) run trimmed 10-row matmuls (N=330 instead of 363); phase-2
   chunk order is rotated so a full-width chunk opens each accumulation
   group (every element's first writer is the bank-clearing start matmul);
 - bf16 output tiles (halved copies/DMAs), the last phase writes per-m and
   the final two transfers are split across both HWDGE queues;
 - optional (BASSK_PAIR=1) phases 1+2 fused m-outer/chunk-mid/n-inner so
   consecutive matmuls share lhsT (stationary-operand reuse).
"""

import os
import numpy as np
import ml_dtypes

import concourse.bass as bass
import concourse.mybir as mybir
import concourse.tile as tile
from concourse.bass_utils import run_bass_kernel_spmd

PD = 4
C = 16              # channels per core
N_CORES = 8
CHUNKS = [(a, sy, sx) for a in range(8) for sy in (0, 1) for sx in (0, 1)]

NV = 33
NT = 11 * NV          # N per matmul: 11 u-rows x NV v-cols

NSINGLE = 12          # chunks 1..NSINGLE-1 stream as single-chunk DMAs

TRIM = os.environ.get("BASSK_TRIM", "1") == "1"
PAIR = os.environ.get("BASSK_PAIR", "1") == "1"
WARM = int(os.environ.get("BASSK_WARM", "14"))


def _split_multi_sync(nc):
    """The walrus in this env allows only ONE sync-wait per instruction.
    Hoist extra waits onto same-engine InstNoOp carriers placed just before
    the owning instruction (sequential waits on one engine == AND)."""
    ctr = 0
    for f in nc.m.functions:
        for bb in f.blocks:
            insts = list(bb.instructions)
            out = []
            changed = False
            for inst in insts:
                si = inst.sync_info
                waits = list(si.on_wait) if si and si.on_wait else []
                if len(waits) > 1:
                    for w in waits[:-1]:
                        nop = mybir.InstNoOp(name=f"waitnop-{ctr}", ins=[], outs=[])
                        ctr += 1
                        nop.engine = inst.engine
                        nop.sync_info = mybir.SyncInfo(on_wait=[w], on_update=[])
                        out.append(nop)
                    si.on_wait = [waits[-1]]
                    changed = True
                out.append(inst)
            if changed:
                bb.instructions = out
    return ctr


def _mm_rows(n, sy):
    """(row_lo, row_hi) within the 11-row phase window; trims the row that
    reads the all-zero X padding (u'=0 when n==0,sy==1; u'=10 when
    n==2,sy==0)."""
    lo, hi = 0, 11
    if TRIM and n == 0 and sy == 1:
        lo = 1
    if TRIM and n == 2 and sy == 0:
        hi = 10
    return lo, hi


def _build_nc():
    f32 = mybir.dt.float32
    bf16 = mybir.dt.bfloat16
    nc = bass.Bass(enable_partition_id=False)
    # W chunks pre-gathered host-side, partition-major: [p, ci, (c,ry,rx)].
    w4 = nc.declare_dram_parameter("w4", [128, 32, C * 64], bf16, isOutput=False)
    mT = nc.declare_dram_parameter("mT", [128, 32, 64], bf16, isOutput=False)
    # X is y-major [p, yy, a, xx] and loads in 3 phase-aligned y-slabs:
    # phase n only reads rows [11n, 11n+13).
    xp = nc.declare_dram_parameter("xp", [128, 34, 8, 36], bf16, isOutput=False)
    # a-major copy of slab-0 rows so the startup pieces are contiguous
    xp0 = nc.declare_dram_parameter("xp0", [128, 8, 13, 36], bf16, isOutput=False)
    # out: phase-major [n, p, m, NT]
    outT = nc.declare_dram_parameter("outT", [3, 128, 8, NT], bf16, isOutput=True)

    with tile.TileContext(nc) as tc:
        with (
            tc.tile_pool(name="dzp", bufs=1) as dzp,
            tc.tile_pool(name="xpp", bufs=1) as xpp,
            tc.tile_pool(name="wp", bufs=1) as wp,
            tc.tile_pool(name="mp", bufs=1) as mp,
            tc.tile_pool(name="op", bufs=6) as op,
            tc.tile_pool(name="pp", bufs=8, space="PSUM") as pp,
        ):
            # --- PE warm-up bridge: the HAM clock gate opens only after
            # ~3.4us of sustained PE activity.  These matmuls depend only on
            # a local memset, so they start as soon as the engines clear the
            # preamble (~7.5us) and keep the PE busy until the first real MM;
            # by then the 2.4GHz clock is (nearly) un-throttled.  warm_ps is
            # slot 0 of the "ps" ring and doubles as phase-0 m=0's bank (the
            # real group's start=True matmul clears it).
            warm_ps = pp.tile([128, NT], f32, tag="ps", name="warm_ps")
            if WARM:
                dz = dzp.tile([128, 128], bf16, name="dz")
                nc.gpsimd.memset(dz[:], 0)
                for _ in range(WARM):
                    nc.tensor.matmul(warm_ps[:, 0:128], dz[:], dz[:],
                                     start=True, stop=True)

            # --- Startup-critical DMAs lead each queue's FIFO.
            # sync queue:   mask col 0, chunk-0 W slices, then W per-chunk.
            # scalar queue: X slab-0 pieces + remaining masks, then X slabs
            #               (deferred) and outT writebacks.
            mtc0 = mp.tile([128, 1, 64], bf16, name="mtc0")
            nc.sync.dma_start(mtc0[:], mT[:, 0:1])
            wc0a = wp.tile([128, 128], bf16, name="wc0a")
            nc.sync.dma_start(wc0a[:], w4[:, 0, 0:128])
            wc0b = wp.tile([128, 384], bf16, name="wc0b")
            nc.sync.dma_start(wc0b[:], w4[:, 0, 128:512])
            wc0c = wp.tile([128, 512], bf16, name="wc0c")
            nc.sync.dma_start(wc0c[:], w4[:, 0, 512:1024])

            s0a = xpp.tile([128, 1, 13, 36], bf16, name="s0a")
            nc.scalar.dma_start(s0a[:], xp0[:, 0:1])
            mta0b = mp.tile([128, 3, 64], bf16, name="mta0b")
            nc.scalar.dma_start(mta0b[:], mT[:, 1:4])
            s0b = xpp.tile([128, 1, 13, 36], bf16, name="s0b")
            nc.scalar.dma_start(s0b[:], xp0[:, 1:2])
            mta1 = mp.tile([128, 28, 64], bf16, name="mta1")
            nc.scalar.dma_start(mta1[:], mT[:, 4:32])
            s0c = xpp.tile([128, 6, 13, 36], bf16, name="s0c")
            nc.scalar.dma_start(s0c[:], xp0[:, 2:8])

            # W chunks 1..11 as single-chunk DMAs (the early stream must land
            # at PE cadence; a whole-quad DMA's semaphore only fires when all
            # 1MB landed), chunks 12..31 as quads.
            cw = [None]
            for k in range(1, NSINGLE):
                t = wp.tile([128, C * 64], bf16, name=f"cw_{k}")
                nc.sync.dma_start(t[:], w4[:, k])
                cw.append(t)
            quads = {}
            for g in range(NSINGLE // 4, 8):
                wq = wp.tile([128, 4, C * 64], bf16, name=f"wq_{g}")
                nc.sync.dma_start(wq[:], w4[:, 4 * g:4 * g + 4, :])
                quads[g] = wq

            # DVE FIFO order matters: critical ops first, then per-chunk
            # mask-muls in consumption order with the 1-m ops just ahead of
            # first use.
            nc.vector.tensor_scalar(
                out=mtc0[:], in0=mtc0[:], scalar1=-1.0, scalar2=1.0,
                op0=mybir.AluOpType.mult, op1=mybir.AluOpType.add,
            )

            def mask_mul(w_ap, ci, nch):
                m = (mtc0[:, 0, :] if ci == 0
                     else mta0b[:, ci - 1, :] if ci < 4
                     else mta1[:, ci - 4, :])
                wv = w_ap.rearrange("p (c f) -> p c f", c=nch)
                nc.vector.tensor_tensor(
                    out=wv, in0=wv,
                    in1=m[:, None, :].broadcast_to([128, nch, 64]),
                    op=mybir.AluOpType.mult,
                )

            mask_mul(wc0a[:], 0, 2)
            mask_mul(wc0b[:], 0, 6)
            mask_mul(wc0c[:], 0, 8)
            nc.vector.tensor_scalar(
                out=mta0b[:], in0=mta0b[:], scalar1=-1.0, scalar2=1.0,
                op0=mybir.AluOpType.mult, op1=mybir.AluOpType.add,
            )
            for k in range(1, 4):
                mask_mul(cw[k][:], k, C)
            nc.vector.tensor_scalar(
                out=mta1[:], in0=mta1[:], scalar1=-1.0, scalar2=1.0,
                op0=mybir.AluOpType.mult, op1=mybir.AluOpType.add,
            )
            for k in range(4, NSINGLE):
                mask_mul(cw[k][:], k, C)
            for ci in range(NSINGLE, 32):
                g, j = ci // 4, ci % 4
                mask_mul(quads[g][:, j, :], ci, C)

            def lhsT(ci, m):
                if ci == 0:
                    if m == 0:
                        return wc0a[:]
                    if m < 4:
                        return wc0b[:, 128 * (m - 1):128 * m]
                    return wc0c[:, 128 * (m - 4):128 * (m - 3)]
                if ci < NSINGLE:
                    return cw[ci][:, 128 * m:128 * (m + 1)]
                return quads[ci // 4][:, ci % 4, 128 * m:128 * (m + 1)]

            SLABS = ((0, 13), (11, 24), (22, 34))
            slab_dmas = [None]
            slabs = [None]
            for si, (y0, y1) in list(enumerate(SLABS))[1:]:
                st = xpp.tile([128, y1 - y0, 8, 36], bf16, name=f"slab_{si}")
                slab_dmas.append(nc.scalar.dma_start(st[:], xp[:, y0:y1]))
                slabs.append(st)

            def rhs_ap(n, ci):
                a, sy, sx = CHUNKS[ci]
                lo, hi = _mm_rows(n, sy)
                y0 = 11 * n + 1 - sy - SLABS[n][0]
                x0 = 1 - sx
                if n == 0:
                    t, aa = ((s0a, 0) if a == 0 else
                             (s0b, 0) if a == 1 else (s0c, a - 2))
                    return t[:, aa, y0 + lo:y0 + hi, x0:x0 + NV]
                return slabs[n][:, y0 + lo:y0 + hi, a, x0:x0 + NV]

            def ps_ap(ps, n, ci):
                _, sy, _ = CHUNKS[ci]
                lo, hi = _mm_rows(n, sy)
                return ps[:, NV * lo:NV * hi]

            # Phase 2 accumulation order: a full-width (sy=1 when trimming)
            # chunk must open each group so the start=True matmul covers the
            # whole psum width.
            order2 = ([ci for ci in range(32) if CHUNKS[ci][1] == 1]
                      + [ci for ci in range(32) if CHUNKS[ci][1] == 0]) \
                if TRIM else list(range(32))

            # Phase 0 (n=0) is chunk-outer with 8 live psum groups so the PE
            # consumes W chunks as they stream (no all-32-chunks stall).
            pss = [warm_ps] + [pp.tile([128, NT], f32, tag="ps", name=f"ps_0_{i}")
                               for i in range(1, 8)]
            mm0 = {}
            for ci in range(32):
                rhs = rhs_ap(0, ci)
                for m in range(8):
                    mm0[ci, m] = nc.tensor.matmul(
                        ps_ap(pss[m], 0, ci), lhsT(ci, m), rhs,
                        start=(ci == 0), stop=(ci == 31),
                    )
            # Defer the X slabs for phases 1-2 behind PE progress so the
            # startup-critical stream gets the full DMA bandwidth during ramp.
            from concourse.tile_rust import add_dep_helper
            add_dep_helper(slab_dmas[1].ins, mm0[3, 7].ins,
                           sync=True, reason="slab1 after early phase0")
            add_dep_helper(slab_dmas[2].ins, mm0[11, 7].ins,
                           sync=True, reason="slab2 after mid phase0")
            # per-2m writeback tiles: whole-tile dependency tracking means one
            # big tile would hold every outT DMA until the LAST psum copy.
            for k in range(4):
                ot = op.tile([128, 2, NT], bf16, tag="o", name=f"osb_0_{k}")
                nc.vector.tensor_copy(ot[:, 0, :], pss[2 * k][:])
                nc.vector.tensor_copy(ot[:, 1, :], pss[2 * k + 1][:])
                nc.scalar.dma_start(outT[0, :, 2 * k:2 * k + 2], ot[:])

            if PAIR:
                # Phases 1+2 fused: m-outer, chunk-mid, n-inner.  The two
                # matmuls of a chunk share lhsT (stationary operand stays
                # loaded) and land in two psum banks.  All X is resident.
                for m in range(8):
                    ps1 = pp.tile([128, NT], f32, tag="ps", name=f"ps_12_{m}a")
                    ps2 = pp.tile([128, NT], f32, tag="ps", name=f"ps_12_{m}b")
                    for idx, ci in enumerate(order2):
                        st, sp = (idx == 0), (idx == 31)
                        w = lhsT(ci, m)
                        nc.tensor.matmul(ps_ap(ps1, 1, ci), w, rhs_ap(1, ci),
                                         start=st, stop=sp)
                        nc.tensor.matmul(ps_ap(ps2, 2, ci), w, rhs_ap(2, ci),
                                         start=st, stop=sp)
                    if m < 7:
                        ot = op.tile([128, 2, NT], bf16, tag="o",
                                     name=f"osb_12_{m}")
                        nc.vector.tensor_copy(ot[:, 0, :], ps1[:])
                        nc.vector.tensor_copy(ot[:, 1, :], ps2[:])
                        nc.scalar.dma_start(
                            outT[1:3, :, m:m + 1].rearrange("n p o f -> p (n o) f"),
                            ot[:])
                    else:
                        # split the tail across both queues so the final
                        # landing + completion receipt is short
                        o1 = op.tile([128, 1, NT], bf16, tag="o", name="osb_t1")
                        nc.vector.tensor_copy(o1[:, 0, :], ps1[:])
                        nc.scalar.dma_start(outT[1, :, m:m + 1], o1[:])
                        o2 = op.tile([128, 1, NT], bf16, tag="o", name="osb_t2")
                        nc.vector.tensor_copy(o2[:, 0, :], ps2[:])
                        nc.sync.dma_start(outT[2, :, m:m + 1], o2[:])
            else:
                for n in (1, 2):
                    order = order2 if n == 2 else list(range(32))
                    ot = None
                    for m in range(8):
                        ps = pp.tile([128, NT], f32, tag="ps", name=f"ps_{n}_{m}")
                        for idx, ci in enumerate(order):
                            nc.tensor.matmul(
                                ps_ap(ps, n, ci), lhsT(ci, m), rhs_ap(n, ci),
                                start=(idx == 0), stop=(idx == 31),
                            )
                        if n == 2 and m >= 6:
                            o1 = op.tile([128, 1, NT], bf16, tag="o",
                                         name=f"osb_t{m}")
                            nc.vector.tensor_copy(o1[:, 0, :], ps[:])
                            eng = nc.scalar if m == 6 else nc.sync
                            eng.dma_start(outT[n, :, m:m + 1], o1[:])
                        else:
                            if m % 2 == 0:
                                ot = op.tile([128, 2, NT], bf16, tag="o",
                                             name=f"osb_{n}_{m // 2}")
                            nc.vector.tensor_copy(ot[:, m % 2, :], ps[:])
                            if m % 2 == 1:
                                nc.scalar.dma_start(
                                    outT[n, :, m - 1:m + 1], ot[:])

    _split_multi_sync(nc)
    return nc


def _host_prep(b_ch, mask_b, cos_b):
    """b_ch (16,256,256) f32, mask_b (256,256) f32, cos_b (1024,32,32) f32
    -> dict of device inputs (layout/gather glue only)."""
    bpad = np.pad(b_ch, ((0, 0), (PD, PD), (PD, PD)), mode="edge")
    mpad = np.pad(mask_b, ((PD, PD), (PD, PD)), mode="edge")
    # block layout [bi*33+bj, (c,ry,rx)]
    bT = bpad.reshape(C, 33, 8, 33, 8).transpose(1, 3, 0, 2, 4).reshape(33 * 33, C * 64)
    mTb = mpad.reshape(33, 8, 33, 8).transpose(0, 2, 1, 3).reshape(33 * 33, 64)
    # unfold-as-shifted-block-views: chunk (a,sy,sx), partition p=32*pi+pj
    # reads block row (4a+pi+sy)*33 + (pj+sx).  Pre-gather partition-major.
    pi, pj = np.arange(4)[:, None], np.arange(32)[None, :]
    rows = np.stack([((4 * a + pi + sy) * 33 + (pj + sx)).reshape(128)
                     for (a, sy, sx) in CHUNKS], axis=1)        # [128, 32]
    w4 = np.ascontiguousarray(bT[rows]).astype(ml_dtypes.bfloat16)
    mT = np.ascontiguousarray(mTb[rows]).astype(ml_dtypes.bfloat16)
    xp = np.zeros((1024, 34, 36), np.float32)
    xp[:, 1:33, 1:33] = cos_b
    # [l=128a+p, yy, xx] -> [p, yy, a, xx]; plus an a-major slab-0 copy
    xpb = xp.reshape(8, 128, 34, 36)
    xp0 = np.ascontiguousarray(xpb[:, :, 0:13, :].transpose(1, 0, 2, 3))
    xp = np.ascontiguousarray(xpb.transpose(1, 2, 0, 3))
    return {"w4": w4, "mT": mT,
            "xp": xp.astype(ml_dtypes.bfloat16),
            "xp0": xp0.astype(ml_dtypes.bfloat16)}


def _unshard(outT):
    # outT [3, 128, 8, 11*NV] -> [(c,ry,rx)=128m+p, u=11n+u', v] -> (16,256,256)
    outT = np.asarray(outT).astype(np.float32)
    t = outT.reshape(3, 128, 8, 11, NV).transpose(2, 1, 0, 3, 4).reshape(1024, 33, NV)
    t = t[:, :, :33].reshape(C, 8, 8, 33, 33).transpose(0, 3, 1, 4, 2)
    return t.reshape(C, 264, 264)[:, 4:260, 4:260]


_RUN_KW = {}   # test harness may inject e.g. trace=True
_LAST_RESULTS = [None]
_NC_CACHE = {}


def _get_nc():
    key = (TRIM, PAIR, WARM)
    nc = _NC_CACHE.get(key)
    if nc is None:
        nc = _NC_CACHE[key] = _build_nc()
    return nc


def kernel(cos_similar, b, mask):
    cos_similar = np.ascontiguousarray(np.asarray(cos_similar, dtype=np.float32))
    b = np.ascontiguousarray(np.asarray(b, dtype=np.float32))
    mask = np.ascontiguousarray(np.asarray(mask, dtype=np.float32))

    in_maps = []
    for core in range(N_CORES):
        batch, half = core // 2, core % 2
        ch0 = C * half
        in_maps.append(_host_prep(
            b[batch, ch0:ch0 + C], mask[batch, 0], cos_similar[batch]))

    nc = _get_nc()
    res = run_bass_kernel_spmd(nc, in_maps, list(range(N_CORES)), **_RUN_KW)
    _LAST_RESULTS[0] = res

    out = np.empty((4, 32, 256, 256), np.float32)
    for core in range(N_CORES):
        batch, half = core // 2, core % 2
        ch0 = C * half
        out[batch, ch0:ch0 + C] = _unshard(res.results[core]["outT"])
    return out
